# revision 6
# baseline (speedup 1.0000x reference)
"""Category-specific linear (MoE-style routed batched matmul) on 8 trn2 cores.

out[b, s, h] = sum_i x[b, s, i] * W[cat_ids[b], i, h] + bias[cat_ids[b], h]

Shapes (hardcoded): x (32, 512, 1024) f32, cat_ids (32,) int, W (16, 1024, 4096)
f32, b (16, 4096) f32 -> out (32, 512, 4096) f32.

Strategy: data-parallel over batch, 4 batches per core, with host-side routing
that always packs one same-category PAIR of batches plus two singles per core
(slot capacities [2, 1, 1] batches). With 32 batches over 16 categories there
are always >= (32 - 16)/2 = 8 disjoint same-category pairs, so this packing is
feasible for ANY cat_ids. Each core then loads only 3 weight matrices (48 MB)
instead of 4 (64 MB), which moves the kernel from DMA-bound to compute-bound.

Per core (slot-major):
  for slot s in [A(2 batches), B(1), C(1)]:
    for n-half (2 x 2048 cols):
      stream W[s]-half as 8 k-tiles [128, 2048] (1 MB DMAs, sync HWDGE ring)
      for m over the slot's 128-sample tiles (8 for A, 4 for B/C):
        for kt(8) x n4(4): fp32r matmul -> psum[n4] (accumulate over kt)
        evict psum + bias (DVE add) -> sbuf, DMA to out (gpsimd SWDGE)

x.T tiles ride the scalar HWDGE ring (issued first, so the PE starts early);
W gets the sync ring to itself; output stores go on SWDGE.
fp32r runs the PE at 1 cycle/row (4x fp32) with ~1.4e-4 relative error.
"""

import numpy as np

import concourse.bacc as bacc
import concourse.mybir as mybir
import concourse.bass as bass
import concourse.tile as tile
from concourse.bass_utils import run_bass_kernel_spmd

N_CORES = 8
B, S, K, H = 32, 512, 1024, 4096
BPC = B // N_CORES          # batches per core
P = 128                     # partitions
KT = K // P                 # k tiles (8)
MT = S // P                 # sample tiles per batch (4)
NHALF = 2                   # n halves
NH = H // NHALF             # cols per half (2048)
NMM = NH // 512             # 512-wide matmuls per half (4)
SLOT_BATCHES = (2, 1, 1)    # batches per weight slot
NSLOT = len(SLOT_BATCHES)

_COMPILED = None


def _build():
    nc = bacc.Bacc("TRN2", target_bir_lowering=False, debug=False)
    f32 = mybir.dt.float32
    f32r = mybir.dt.float32r

    xt_ap = nc.dram_tensor("xt", [BPC, K, S], f32r, kind="ExternalInput").ap()
    w_ap = nc.dram_tensor("w", [NSLOT, K, H], f32r, kind="ExternalInput").ap()
    bias_ap = nc.dram_tensor("bias", [NSLOT, H], f32, kind="ExternalInput").ap()
    out_ap = nc.dram_tensor("out", [BPC, S, H], f32, kind="ExternalOutput").ap()

    with tile.TileContext(nc) as tc:
        with (
            tc.tile_pool(name="xt_pool", bufs=8) as xt_pool,
            tc.tile_pool(name="w_pool", bufs=16) as w_pool,
            tc.tile_pool(name="bias_pool", bufs=2) as bias_pool,
            tc.tile_pool(name="out_pool", bufs=3) as out_pool,
            tc.tile_pool(name="ps_pool", bufs=8, space="PSUM") as ps_pool,
        ):
            bi0 = 0  # first batch index of this slot
            for s in range(NSLOT):
                nb = SLOT_BATCHES[s]
                # xt tiles per (batch, m): [128, kt, 128], 512 KB each, so the
                # first matmul group only waits on one small transfer.
                xt_ts = {}
                for b in range(nb):
                    src = xt_ap[bi0 + b].rearrange("(kt p) m -> p kt m", p=P)
                    for mm in range(MT):
                        xt_t = xt_pool.tile([P, KT, P], f32r, name="xt_t", tag="xt")
                        nc.scalar.dma_start(
                            xt_t[:], src[:, :, mm * P : (mm + 1) * P]
                        )
                        xt_ts[(b, mm)] = xt_t
                w_r = w_ap[s].rearrange("(kt p) n -> p kt n", p=P)
                for half in range(NHALF):
                    # bias half, broadcast across partitions: [128, 2048]
                    bias_t = bias_pool.tile([P, NH], f32, name="bias_t")
                    bias_src = bias_ap[s, half * NH : (half + 1) * NH]
                    nc.scalar.dma_start(
                        out=bias_t[:],
                        in_=bass.AP(
                            tensor=bias_src.tensor,
                            offset=bias_src.offset,
                            ap=[[0, P]] + list(bias_src.ap),
                        ),
                    )
                    w_tiles = []
                    for kt in range(KT):
                        w_t = w_pool.tile([P, NH], f32r, tag="w", name="w_t")
                        nc.sync.dma_start(
                            w_t[:], w_r[:, kt, half * NH : (half + 1) * NH]
                        )
                        w_tiles.append(w_t)
                    for m in range(nb * MT):
                        b, mm = divmod(m, MT)
                        ps = [
                            ps_pool.tile([P, 512], f32, tag="ps", name="ps")
                            for _ in range(NMM)
                        ]
                        for kt in range(KT):
                            lhsT = xt_ts[(b, mm)][:, kt, :]
                            for n4 in range(NMM):
                                nc.tensor.matmul(
                                    ps[n4][:],
                                    lhsT,
                                    w_tiles[kt][:, n4 * 512 : (n4 + 1) * 512],
                                    start=(kt == 0),
                                    stop=(kt == KT - 1),
                                )
                        out_t = out_pool.tile([P, NH], f32)
                        for n4 in range(NMM):
                            nc.vector.tensor_add(
                                out_t[:, n4 * 512 : (n4 + 1) * 512],
                                ps[n4][:],
                                bias_t[:, n4 * 512 : (n4 + 1) * 512],
                            )
                        nc.gpsimd.dma_start(
                            out_ap[
                                bi0 + b,
                                mm * P : (mm + 1) * P,
                                half * NH : (half + 1) * NH,
                            ],
                            out_t[:],
                        )
                bi0 += nb
    nc.compile()
    return nc


def _get_compiled():
    global _COMPILED
    if _COMPILED is None:
        _COMPILED = _build()
    return _COMPILED


def _pack(cat_ids):
    """Assign batches to cores with slot capacities [2,1,1] per core.

    Returns per-core (idx, slot_cats): idx = 4 batch indices ordered
    [pair0, pair1, single_b, single_c]; slot_cats = categories for the 3 slots.
    Always feasible: #disjoint same-cat pairs = (32 - #odd-count cats)/2 >= 8.
    """
    cat_ids = np.asarray(cat_ids)
    by_cat = {}
    for i, c in enumerate(cat_ids.tolist()):
        by_cat.setdefault(c, []).append(i)
    pairs = []
    singles = []
    for c, idxs in sorted(by_cat.items()):
        n = len(idxs)
        for j in range(n // 2):
            pairs.append((c, idxs[2 * j], idxs[2 * j + 1]))
        if n % 2:
            singles.append((c, idxs[-1]))
    assert len(pairs) >= N_CORES, "impossible: <8 same-cat pairs among 32 batches"
    core_pairs = pairs[:N_CORES]
    # leftovers: extra pairs flatten into singles
    for c, i, j in pairs[N_CORES:]:
        singles.append((c, i))
        singles.append((c, j))
    assert len(singles) == 2 * N_CORES
    cores = []
    for ci in range(N_CORES):
        c, i, j = core_pairs[ci]
        (cb, ib), (cc, ic) = singles[2 * ci], singles[2 * ci + 1]
        cores.append(([i, j, ib, ic], [c, cb, cc]))
    return cores


def run_sharded(x, cat_ids, W, b, trace=False, **spmd_kwargs):
    """Shard, run on 8 cores, unshard. Returns (out, BassKernelResults)."""
    x = np.ascontiguousarray(np.asarray(x), dtype=np.float32)
    cat_ids = np.asarray(cat_ids).astype(np.int64)
    W = np.ascontiguousarray(np.asarray(W), dtype=np.float32)
    b = np.ascontiguousarray(np.asarray(b), dtype=np.float32)

    nc = _get_compiled()
    cores = _pack(cat_ids)

    in_maps = []
    for idx, slot_cats in cores:
        in_maps.append(
            {
                "xt": np.ascontiguousarray(x[idx].transpose(0, 2, 1)),
                "w": np.ascontiguousarray(W[slot_cats]),
                "bias": np.ascontiguousarray(b[slot_cats]),
            }
        )

    res = run_bass_kernel_spmd(
        nc, in_maps, list(range(N_CORES)), trace=trace, **spmd_kwargs
    )

    out = np.empty((B, S, H), dtype=np.float32)
    for c, (idx, _) in enumerate(cores):
        out[idx] = res.results[c]["out"]
    return out, res


def kernel(x, cat_ids, W, b):
    out, _ = run_sharded(x, cat_ids, W, b)
    return out


# revision 7
# speedup vs baseline: 1.0822x; 1.0822x over previous
"""Category-specific linear (MoE-style routed batched matmul) on 8 trn2 cores.

out[b, s, h] = sum_i x[b, s, i] * W[cat_ids[b], i, h] + bias[cat_ids[b], h]

Shapes (hardcoded): x (32, 512, 1024) f32, cat_ids (32,) int, W (16, 1024, 4096)
f32, b (16, 4096) f32 -> out (32, 512, 4096) f32.

Strategy: data-parallel over batch, 4 batches per core, with host-side routing
that always packs one same-category PAIR of batches plus two singles per core
(slot capacities [2, 1, 1] batches). With 32 batches over 16 categories there
are always >= (32 - 16)/2 = 8 disjoint same-category pairs, so this packing is
feasible for ANY cat_ids. Each core then loads only 3 weight matrices (48 MB)
instead of 4 (64 MB), which moves the kernel from DMA-bound to compute-bound.

Per core (slot-major):
  for slot s in [A(2 batches), B(1), C(1)]:
    for n-half (2 x 2048 cols):
      stream W[s]-half as 8 k-tiles [128, 2048] (1 MB DMAs, sync HWDGE ring)
      for m over the slot's 128-sample tiles (8 for A, 4 for B/C):
        for kt(8) x n4(4): fp32r matmul -> psum[n4] (accumulate over kt)
        evict psum + bias (DVE add) -> sbuf, DMA to out (scalar HWDGE)

x.T tiles ride the scalar HWDGE ring (issued first, so the PE starts early),
followed by bias and output stores; W gets the sync ring to itself.
fp32r runs the PE at 1 cycle/row (4x fp32) with ~1.4e-4 relative error.
"""

import numpy as np

import concourse.bacc as bacc
import concourse.mybir as mybir
import concourse.bass as bass
import concourse.tile as tile
from concourse.bass_utils import run_bass_kernel_spmd

N_CORES = 8
B, S, K, H = 32, 512, 1024, 4096
BPC = B // N_CORES          # batches per core
P = 128                     # partitions
KT = K // P                 # k tiles (8)
MT = S // P                 # sample tiles per batch (4)
NHALF = 2                   # n halves
NH = H // NHALF             # cols per half (2048)
NMM = NH // 512             # 512-wide matmuls per half (4)
SLOT_BATCHES = (2, 1, 1)    # batches per weight slot
NSLOT = len(SLOT_BATCHES)

_COMPILED = None


def _build():
    nc = bacc.Bacc("TRN2", target_bir_lowering=False, debug=False)
    f32 = mybir.dt.float32
    f32r = mybir.dt.float32r

    xt_ap = nc.dram_tensor("xt", [BPC, K, S], f32r, kind="ExternalInput").ap()
    w_ap = nc.dram_tensor("w", [NSLOT, K, H], f32r, kind="ExternalInput").ap()
    bias_ap = nc.dram_tensor("bias", [NSLOT, H], f32, kind="ExternalInput").ap()
    out_ap = nc.dram_tensor("out", [BPC, S, H], f32, kind="ExternalOutput").ap()

    with tile.TileContext(nc) as tc:
        with (
            tc.tile_pool(name="xt_pool", bufs=3) as xt_pool,
            tc.tile_pool(name="w_pool", bufs=14) as w_pool,
            tc.tile_pool(name="bias_pool", bufs=2) as bias_pool,
            tc.tile_pool(name="out_pool", bufs=3) as out_pool,
            tc.tile_pool(name="ps_pool", bufs=8, space="PSUM") as ps_pool,
        ):
            bi0 = 0  # first batch index of this slot
            for s in range(NSLOT):
                nb = SLOT_BATCHES[s]
                # xt tiles per batch: [128, kt, 512] (2 MB, 2 KB chunks),
                # issued first on the scalar ring so the PE starts early.
                xt_ts = []
                for b in range(nb):
                    xt_t = xt_pool.tile([P, KT, S], f32r, name="xt_t", tag="xt")
                    nc.scalar.dma_start(
                        xt_t[:], xt_ap[bi0 + b].rearrange("(kt p) m -> p kt m", p=P)
                    )
                    xt_ts.append(xt_t)
                w_r = w_ap[s].rearrange("(kt p) n -> p kt n", p=P)
                for half in range(NHALF):
                    # bias half, broadcast across partitions: [128, 2048]
                    bias_t = bias_pool.tile([P, NH], f32, name="bias_t")
                    bias_src = bias_ap[s, half * NH : (half + 1) * NH]
                    nc.scalar.dma_start(
                        out=bias_t[:],
                        in_=bass.AP(
                            tensor=bias_src.tensor,
                            offset=bias_src.offset,
                            ap=[[0, P]] + list(bias_src.ap),
                        ),
                    )
                    w_tiles = []
                    for kt in range(KT):
                        w_t = w_pool.tile([P, NH], f32r, tag="w", name="w_t")
                        nc.sync.dma_start(
                            w_t[:], w_r[:, kt, half * NH : (half + 1) * NH]
                        )
                        w_tiles.append(w_t)
                    for m in range(nb * MT):
                        b, mm = divmod(m, MT)
                        ps = [
                            ps_pool.tile([P, 512], f32, tag="ps", name="ps")
                            for _ in range(NMM)
                        ]
                        for kt in range(KT):
                            lhsT = xt_ts[b][:, kt, mm * P : (mm + 1) * P]
                            for n4 in range(NMM):
                                nc.tensor.matmul(
                                    ps[n4][:],
                                    lhsT,
                                    w_tiles[kt][:, n4 * 512 : (n4 + 1) * 512],
                                    start=(kt == 0),
                                    stop=(kt == KT - 1),
                                )
                        out_t = out_pool.tile([P, NH], f32)
                        for n4 in range(NMM):
                            nc.vector.tensor_add(
                                out_t[:, n4 * 512 : (n4 + 1) * 512],
                                ps[n4][:],
                                bias_t[:, n4 * 512 : (n4 + 1) * 512],
                            )
                        nc.scalar.dma_start(
                            out_ap[
                                bi0 + b,
                                mm * P : (mm + 1) * P,
                                half * NH : (half + 1) * NH,
                            ],
                            out_t[:],
                        )
                bi0 += nb
    nc.compile()
    return nc


def _get_compiled():
    global _COMPILED
    if _COMPILED is None:
        _COMPILED = _build()
    return _COMPILED


def _pack(cat_ids):
    """Assign batches to cores with slot capacities [2,1,1] per core.

    Returns per-core (idx, slot_cats): idx = 4 batch indices ordered
    [pair0, pair1, single_b, single_c]; slot_cats = categories for the 3 slots.
    Always feasible: #disjoint same-cat pairs = (32 - #odd-count cats)/2 >= 8.
    """
    cat_ids = np.asarray(cat_ids)
    by_cat = {}
    for i, c in enumerate(cat_ids.tolist()):
        by_cat.setdefault(c, []).append(i)
    pairs = []
    singles = []
    for c, idxs in sorted(by_cat.items()):
        n = len(idxs)
        for j in range(n // 2):
            pairs.append((c, idxs[2 * j], idxs[2 * j + 1]))
        if n % 2:
            singles.append((c, idxs[-1]))
    assert len(pairs) >= N_CORES, "impossible: <8 same-cat pairs among 32 batches"
    core_pairs = pairs[:N_CORES]
    # leftovers: extra pairs flatten into singles
    for c, i, j in pairs[N_CORES:]:
        singles.append((c, i))
        singles.append((c, j))
    assert len(singles) == 2 * N_CORES
    cores = []
    for ci in range(N_CORES):
        c, i, j = core_pairs[ci]
        (cb, ib), (cc, ic) = singles[2 * ci], singles[2 * ci + 1]
        cores.append(([i, j, ib, ic], [c, cb, cc]))
    return cores


def run_sharded(x, cat_ids, W, b, trace=False, **spmd_kwargs):
    """Shard, run on 8 cores, unshard. Returns (out, BassKernelResults)."""
    x = np.ascontiguousarray(np.asarray(x), dtype=np.float32)
    cat_ids = np.asarray(cat_ids).astype(np.int64)
    W = np.ascontiguousarray(np.asarray(W), dtype=np.float32)
    b = np.ascontiguousarray(np.asarray(b), dtype=np.float32)

    nc = _get_compiled()
    cores = _pack(cat_ids)

    in_maps = []
    for idx, slot_cats in cores:
        in_maps.append(
            {
                "xt": np.ascontiguousarray(x[idx].transpose(0, 2, 1)),
                "w": np.ascontiguousarray(W[slot_cats]),
                "bias": np.ascontiguousarray(b[slot_cats]),
            }
        )

    res = run_bass_kernel_spmd(
        nc, in_maps, list(range(N_CORES)), trace=trace, **spmd_kwargs
    )

    out = np.empty((B, S, H), dtype=np.float32)
    for c, (idx, _) in enumerate(cores):
        out[idx] = res.results[c]["out"]
    return out, res


def kernel(x, cat_ids, W, b):
    out, _ = run_sharded(x, cat_ids, W, b)
    return out
